# revision 25
# baseline (speedup 1.0000x reference)
"""Trainium2 Bass kernel for nn_BaselineNCA (dense_cnn, memory-bound).

Network (per image):
    y   = perchannel_conv(x, filters)          # 4 fixed 3x3 filters, circular pad
    hid = relu(w1 @ y + b1)                    # 16 -> 6 channels (1x1 conv)
    y_u = tanh(w2 @ hid + b2)                  # 6 -> 4
    g   = sigmoid(w3 @ hid + b3)               # 6 -> 4
    out = x*g + (1-g)*y_u

Strategy: pure data parallel, 2 images per core on 8 cores.  The
per-channel conv + first 1x1 conv fold into one 4->6ch 3x3 conv whose
weights are baked host-side into banded stationary matrices; the H
direction of the conv and the channel contraction both run on the
TensorEngine (float32r, single-pass) as PSUM-accumulated matmuls, one
per kx shift, reading the same SBUF window at +-1 column offsets.  x is
circularly pre-padded on the host so device DMAs are simple strided
reads.  Elementwise work (relu/sigmoid on ScalarE, gated blend on
VectorE) is merged over pairs of blocks to amortize per-op overhead;
GpSimd is never used (its SW-emulated tensor ops are ~9us and hold the
DVE shared SBUF port).

Layouts (per 21-output-row block, W=512 free dim):
    window  [92,514]  p = h*4+c, h=0..20 aligned rows, p84..88 row -1,
                      p88..92 row 21 (halo rows parked after the aligned
                      rows so blend operands share partition base 0)
    psum1   [126,N]   m = ho*6+o   (hidden pre-act)
    psum3   [84,N]    m = ho*4+c   (gate / y_upd pre-act)
    out     [84,N]    m = ho*4+c
"""

import numpy as np

B, C, H, W = 16, 4, 512, 512
N_CORES = 8
IMG_PER_CORE = B // N_CORES
HO = 21                 # output rows per block
NBLK = 25               # 24 full blocks + 1 block of 8 valid rows
LAST_ROWS = H - (NBLK - 1) * HO  # 8
CHUNK = 5               # blocks per DMA chunk
NCHUNK = NBLK // CHUNK  # 5
GROUPS_BY_SZ = {1: [(0, 1)], 4: [(0, 2), (2, 2)],
                5: [(0, 2), (2, 2), (4, 1)]}  # (start, nblocks) merge groups
HP = H + 15             # padded rows: 1 top + 14 bottom wrap
WP = W + 2              # padded cols
O_HID = 6
P_WIN = 92              # 23 window rows * 4 ch
P_HID = HO * O_HID      # 126
P_OUT = HO * C          # 84

_PROG_CACHE: dict = {}


def _patch_ldw_opt():
    """walrus ships with --enable-ldw-opt=false hardcoded; redundant
    LDWEIGHTS (same stationary back-to-back) pace the PE here.  Rewrite
    the flag on the walrus command line.  Correctness is re-verified
    against the reference output shape/values on every run."""
    import concourse.bass_utils as bu

    if getattr(bu, "_ldw_opt_patched", False):
        return
    orig = bu.run_command

    def run_command(argv, **kwargs):
        argv = [
            "--enable-ldw-opt=true" if a == "--enable-ldw-opt=false" else a
            for a in argv
        ]
        return orig(argv, **kwargs)

    bu.run_command = run_command
    bu._ldw_opt_patched = True


def _build_program(gate_only: bool):
    import concourse.bacc as bacc
    import concourse.bass as bass
    import concourse.mybir as mybir
    from concourse import tile

    # note: _patch_ldw_opt() helps fp32/f32r builds but is incompatible
    # with the standalone LDWEIGHTS that bf16 matmuls emit; bf16 weight
    # loads use FWL and are cheap, so leave ldw-opt off here.

    f32 = mybir.dt.float32
    f32r = mybir.dt.float32r
    AF = mybir.ActivationFunctionType
    ALU = mybir.AluOpType

    nc = bacc.Bacc(None, target_bir_lowering=False)

    # x pre-padded AND pre-transposed on host to [img, row, c, w] so the
    # window DMAs are single partition-contiguous transfers
    bf16 = mybir.dt.bfloat16
    # conv path in f32r (full-rate single-pass matmul, near-fp32 precision);
    # stage-3 (gate) path in bf16
    xp_d = nc.dram_tensor("xp", [IMG_PER_CORE, HP, C, WP], f32r, kind="ExternalInput")
    bm_d = nc.dram_tensor("bmat", [P_WIN, 3 * P_HID], f32r, kind="ExternalInput")
    cw = 84 if gate_only else 168
    wm_d = nc.dram_tensor("wmat", [P_HID, cw], bf16, kind="ExternalInput")
    bias_d = nc.dram_tensor("biases", [P_HID, 4], f32, kind="ExternalInput")
    # output in [img, row, c, w]; host transposes back to [img, c, row, w]
    out_d = nc.dram_tensor("out", [IMG_PER_CORE, H, C, W], f32, kind="ExternalOutput")

    ps_bufs = 2 if gate_only else 1

    with tile.TileContext(nc) as tc:
        with (
            tc.tile_pool(name="consts", bufs=1) as cpool,
            tc.tile_pool(name="win", bufs=4) as win_pool,
            tc.tile_pool(name="outp", bufs=4) as out_pool,
            tc.tile_pool(name="hid", bufs=4) as hid_pool,
            tc.tile_pool(name="gp", bufs=4) as g_pool,
            tc.tile_pool(name="ps1", bufs=ps_bufs, space=bass.MemorySpace.PSUM) as ps1_pool,
            tc.tile_pool(name="ps3", bufs=ps_bufs, space=bass.MemorySpace.PSUM) as ps3_pool,
        ):
            btile = cpool.tile([P_WIN, 3 * P_HID], f32r)
            nc.scalar.dma_start(out=btile[:, :], in_=bm_d[:, :])
            wmt = cpool.tile([P_HID, cw], bf16)
            nc.scalar.dma_start(out=wmt[:, :], in_=wm_d[:, :])
            biast = cpool.tile([P_HID, 4], f32)
            nc.scalar.dma_start(out=biast[:, :], in_=bias_d[:, :])
            bt = btile[0:P_WIN, 0 : 3 * P_HID]
            w3t = wmt[0:P_HID, 0:84]
            b1t = biast[0:P_HID, 0:1]
            b3t = biast[0:P_OUT, 1:2]
            tct = biast[0:P_OUT, 2:3]
            if not gate_only:
                w2t = wmt[0:P_HID, 84:168]
                b2t = biast[0:P_OUT, 3:4]

            for i in range(IMG_PER_CORE):
                sizes = [1, 4, 5, 5, 5, 5] if i == 0 else [5] * 5
                b0 = 0
                for k, sz in enumerate(sizes):
                    r0 = HO * b0  # first output row of chunk
                    is_last = b0 + sz == NBLK
                    win = win_pool.tile([P_WIN, CHUNK * WP], f32r)
                    win3 = win.rearrange("p (b w) -> p b w", w=WP)
                    # aligned rows h=0..20 of each block <- padded rows 21b+1+h
                    # dst partitions 0..84 contiguous; src (h c) rows merge
                    nc.sync.dma_start(
                        out=win3[0:P_OUT, 0:sz, :],
                        in_=xp_d[i, r0 + 1 : r0 + 1 + HO * sz, :, :].rearrange(
                            "(b h) c w -> (h c) b w", h=HO
                        ),
                    )
                    # halo row -1 (slot 21, p84..88) <- padded row 21b
                    eng1 = nc.gpsimd if (i == 0 and k == 0) else nc.sync
                    eng2 = nc.scalar if (i == 0 and k == 0) else nc.sync
                    eng1.dma_start(
                        out=win3[P_OUT : P_OUT + 4, 0:sz, :],
                        in_=xp_d[i, r0 : r0 + (sz - 1) * HO + 1 : HO, :, :].rearrange(
                            "b c w -> c b w"
                        ),
                    )
                    # halo row 21 (slot 22, p88..92) <- padded row 21b+22
                    eng2.dma_start(
                        out=win3[P_OUT + 4 : P_OUT + 8, 0:sz, :],
                        in_=xp_d[
                            i, r0 + 22 : r0 + 22 + (sz - 1) * HO + 1 : HO, :, :
                        ].rearrange("b c w -> c b w"),
                    )

                    outt = out_pool.tile([P_OUT, CHUNK * W], f32)
                    win_f = win.bitcast(f32)

                    for g0, ng in GROUPS_BY_SZ[sz]:
                        nf = ng * W
                        ps1 = ps1_pool.tile([P_HID, nf], f32, tag="ps1")
                        for t in range(3):  # kx taps; dx = t-1
                            for j in range(ng):
                                nc.tensor.matmul(
                                    ps1[:, j * W : (j + 1) * W],
                                    bt[0:P_WIN, t * P_HID : (t + 1) * P_HID],
                                    win[0:P_WIN, (g0 + j) * WP + t :
                                        (g0 + j) * WP + t + W],
                                    start=(t == 0),
                                    stop=(t == 2),
                                )
                        hid = hid_pool.tile([P_HID, nf], bf16, tag="hid")
                        nc.scalar.activation(
                            hid[:, :], ps1[:, :], AF.Relu, bias=b1t[:, 0:1]
                        )
                        ps3 = ps3_pool.tile([P_OUT, nf], f32, tag="ps3")
                        for j in range(ng):
                            nc.tensor.matmul(
                                ps3[:, j * W : (j + 1) * W], w3t[:, :],
                                hid[:, j * W : (j + 1) * W],
                                start=True, stop=True,
                            )
                        g = g_pool.tile([P_OUT, nf], f32, tag="g")
                        nc.scalar.activation(
                            g[:, :], ps3[:, :], AF.Sigmoid, bias=b3t[:, 0:1]
                        )
                        # x view over the group's blocks: [84, ng, 512]
                        xa = win_f.rearrange("p (b w) -> p b w", w=WP)[
                            0:P_OUT, g0 : g0 + ng, 1 : 1 + W
                        ]
                        o3 = outt.rearrange("p (b w) -> p b w", w=W)[
                            0:P_OUT, g0 : g0 + ng, :
                        ]
                        g3 = g.rearrange("p (b w) -> p b w", w=W)
                        if gate_only:
                            # out = (x - tc)*g + tc     (all on VectorE)
                            nc.vector.scalar_tensor_tensor(
                                o3, xa, tct[:, 0:1], g3,
                                op0=ALU.subtract, op1=ALU.mult,
                            )
                            nc.vector.tensor_scalar(
                                o3, o3, tct[:, 0:1], None, ALU.add
                            )
                        else:
                            ps3b = ps3_pool.tile([P_OUT, nf], f32, tag="ps3")
                            for j in range(ng):
                                nc.tensor.matmul(
                                    ps3b[:, j * W : (j + 1) * W], w2t[:, :],
                                    hid[:, j * W : (j + 1) * W],
                                    start=True, stop=True,
                                )
                            yt = g_pool.tile([P_OUT, nf], f32, tag="yt")
                            nc.scalar.activation(
                                yt[:, :], ps3b[:, :], AF.Tanh, bias=b2t[:, 0:1]
                            )
                            y3 = yt.rearrange("p (b w) -> p b w", w=W)
                            d = g_pool.tile([P_OUT, nf], f32, tag="d")
                            d3 = d.rearrange("p (b w) -> p b w", w=W)
                            nc.vector.tensor_sub(d3, xa, y3)
                            nc.vector.tensor_mul(o3, d3, g3)
                            nc.vector.tensor_add(o3, o3, y3)

                    # store chunk
                    outt3 = outt.rearrange("p (b w) -> p b w", w=W)
                    # out-DMAs go on GpSimd (SWDGE): they wait on the blend,
                    # and on the sync HWDGE ring that wait would block the
                    # NEXT chunk's window-DMA issue and stall the PE.
                    nb = sz if not is_last else sz - 1
                    nc.gpsimd.dma_start(
                        out=out_d[i, r0 : r0 + nb * HO, :, :].rearrange(
                            "(b h) c w -> (h c) b w", h=HO
                        ),
                        in_=outt3[:, 0:nb, :],
                    )
                    if is_last:
                        nc.gpsimd.dma_start(
                            out=out_d[i, r0 + nb * HO :, :, :].rearrange(
                                "h c w -> (h c) w"
                            ),
                            in_=outt[0 : LAST_ROWS * C, nb * W : sz * W],
                        )
                    b0 += sz
    nc.compile()
    return nc


def _host_consts(filters, w1_w, w1_b, w2_w, w2_b, w3_w, w3_b, gate_only):
    filters = np.asarray(filters, np.float64)
    w1_w = np.asarray(w1_w, np.float64)
    # fused 4->6ch 3x3 conv kernel
    kf = np.einsum("ocf,fyx->ocyx", w1_w.reshape(O_HID, C, 4), filters)
    bmats = np.zeros((3, P_WIN, P_HID), np.float64)
    for kx in range(3):
        for ho in range(HO):
            for ky in range(3):
                r = ho - 1 + ky
                h_idx = HO if r == -1 else (HO + 1 if r == HO else r)
                for c in range(C):
                    bmats[kx, h_idx * 4 + c, ho * O_HID :
                          ho * O_HID + O_HID] = kf[:, c, ky, kx]
    w3g = np.zeros((P_HID, P_OUT), np.float64)
    w2g = np.zeros((P_HID, P_OUT), np.float64)
    for ho in range(HO):
        for o in range(O_HID):
            for c in range(C):
                w3g[ho * O_HID + o, ho * 4 + c] = np.asarray(w3_w, np.float64)[c, o]
                w2g[ho * O_HID + o, ho * 4 + c] = np.asarray(w2_w, np.float64)[c, o]
    b1v = np.tile(np.asarray(w1_b, np.float64), HO)
    b3v = np.tile(np.asarray(w3_b, np.float64), HO)
    b2v = np.tile(np.asarray(w2_b, np.float64), HO)
    tcv = np.tanh(b2v)
    import ml_dtypes

    cw = 84 if gate_only else 168
    packed = np.zeros((P_HID, cw), np.float64)
    packed[:, 0:84] = w3g
    if not gate_only:
        packed[:, 84:168] = w2g
    biases = np.zeros((P_HID, 4), np.float64)
    biases[:, 0] = b1v
    biases[0:P_OUT, 1] = b3v
    biases[0:P_OUT, 2] = tcv
    biases[0:P_OUT, 3] = b2v
    return {
        "bmat": np.ascontiguousarray(
            bmats.transpose(1, 0, 2).reshape(P_WIN, 378), np.float32
        ),
        "wmat": np.ascontiguousarray(packed.astype(ml_dtypes.bfloat16)),
        "biases": np.ascontiguousarray(biases, np.float32),
    }


def _run(inputs, trace=False, trace_kwargs=None):
    from concourse.bass_utils import run_bass_kernel_spmd

    x = np.asarray(inputs["x"], np.float32)
    w2_w = np.asarray(inputs["w2_w"], np.float32)
    gate_only = bool(np.all(w2_w == 0.0))
    consts = _host_consts(
        inputs["filters"], inputs["w1_w"], inputs["w1_b"], w2_w,
        inputs["w2_b"], inputs["w3_w"], inputs["w3_b"], gate_only,
    )
    if gate_only not in _PROG_CACHE:
        _PROG_CACHE[gate_only] = _build_program(gate_only)
    nc = _PROG_CACHE[gate_only]

    xpad = np.pad(x, ((0, 0), (0, 0), (1, 14), (1, 1)), mode="wrap")
    xq = np.ascontiguousarray(xpad.transpose(0, 2, 1, 3))  # [B, HP, C, WP]
    in_maps = []
    for i in range(N_CORES):
        m = dict(consts)
        m["xp"] = np.ascontiguousarray(
            xq[i * IMG_PER_CORE : (i + 1) * IMG_PER_CORE]
        )
        in_maps.append(m)
    kw = {}
    if trace:
        kw = {"trace": True, **(trace_kwargs or {})}
    res = run_bass_kernel_spmd(nc, in_maps, list(range(N_CORES)), **kw)
    oq = np.concatenate([res.results[i]["out"] for i in range(N_CORES)], axis=0)
    out = np.ascontiguousarray(oq.transpose(0, 2, 1, 3))  # -> [B, C, H, W]
    return out.astype(np.float32), res


def kernel(**inputs) -> np.ndarray:
    out, _ = _run(inputs, trace=False)
    return out


# revision 26
# speedup vs baseline: 1.1138x; 1.1138x over previous
"""Trainium2 Bass kernel for nn_BaselineNCA (dense_cnn, memory-bound).

Network (per image):
    y   = perchannel_conv(x, filters)          # 4 fixed 3x3 filters, circular pad
    hid = relu(w1 @ y + b1)                    # 16 -> 6 channels (1x1 conv)
    y_u = tanh(w2 @ hid + b2)                  # 6 -> 4
    g   = sigmoid(w3 @ hid + b3)               # 6 -> 4
    out = x*g + (1-g)*y_u

Strategy: pure data parallel, 2 images per core on 8 cores.  The
per-channel conv + first 1x1 conv fold into one 4->6ch 3x3 conv whose
weights are baked host-side into banded stationary matrices; the H
direction of the conv and the channel contraction both run on the
TensorEngine (float32r, single-pass) as PSUM-accumulated matmuls, one
per kx shift, reading the same SBUF window at +-1 column offsets.  x is
circularly pre-padded on the host so device DMAs are simple strided
reads.  Elementwise work (relu/sigmoid on ScalarE, gated blend on
VectorE) is merged over pairs of blocks to amortize per-op overhead;
GpSimd is never used (its SW-emulated tensor ops are ~9us and hold the
DVE shared SBUF port).

Layouts (per 21-output-row block, W=512 free dim):
    window  [92,514]  p = h*4+c, h=0..20 aligned rows, p84..88 row -1,
                      p88..92 row 21 (halo rows parked after the aligned
                      rows so blend operands share partition base 0)
    psum1   [126,N]   m = ho*6+o   (hidden pre-act)
    psum3   [84,N]    m = ho*4+c   (gate / y_upd pre-act)
    out     [84,N]    m = ho*4+c
"""

import numpy as np

B, C, H, W = 16, 4, 512, 512
N_CORES = 8
IMG_PER_CORE = B // N_CORES
HO = 21                 # output rows per block
NBLK = 25               # 24 full blocks + 1 block of 8 valid rows
LAST_ROWS = H - (NBLK - 1) * HO  # 8
CHUNK = 5               # blocks per DMA chunk
NCHUNK = NBLK // CHUNK  # 5
GROUPS_BY_SZ = {1: [(0, 1)], 4: [(0, 2), (2, 2)],
                5: [(0, 2), (2, 2), (4, 1)]}  # (start, nblocks) merge groups
HP = H + 15             # padded rows: 1 top + 14 bottom wrap
WP = W + 2              # padded cols
O_HID = 6
P_WIN = 92              # 23 window rows * 4 ch
P_HID = HO * O_HID      # 126
P_OUT = HO * C          # 84

_PROG_CACHE: dict = {}


def _patch_ldw_opt():
    """walrus ships with --enable-ldw-opt=false hardcoded; redundant
    LDWEIGHTS (same stationary back-to-back) pace the PE here.  Rewrite
    the flag on the walrus command line.  Correctness is re-verified
    against the reference output shape/values on every run."""
    import concourse.bass_utils as bu

    if getattr(bu, "_ldw_opt_patched", False):
        return
    orig = bu.run_command

    def run_command(argv, **kwargs):
        argv = [
            "--enable-ldw-opt=true" if a == "--enable-ldw-opt=false" else a
            for a in argv
        ]
        return orig(argv, **kwargs)

    bu.run_command = run_command
    bu._ldw_opt_patched = True


def _build_program(gate_only: bool):
    import concourse.bacc as bacc
    import concourse.bass as bass
    import concourse.mybir as mybir
    from concourse import tile

    # note: _patch_ldw_opt() helps fp32/f32r builds but is incompatible
    # with the standalone LDWEIGHTS that bf16 matmuls emit; bf16 weight
    # loads use FWL and are cheap, so leave ldw-opt off here.

    f32 = mybir.dt.float32
    f32r = mybir.dt.float32r
    AF = mybir.ActivationFunctionType
    ALU = mybir.AluOpType

    nc = bacc.Bacc(None, target_bir_lowering=False)

    # x pre-padded AND pre-transposed on host to [img, row, c, w] so the
    # window DMAs are single partition-contiguous transfers
    f16 = mybir.dt.float16
    # whole matmul path in fp16: full PE rate + FWL weight loads like
    # bf16, but 10-bit mantissa (values here are O(10) -- no range risk)
    xp_d = nc.dram_tensor("xp", [IMG_PER_CORE, HP, C, WP], f16, kind="ExternalInput")
    bm_d = nc.dram_tensor("bmat", [P_WIN, 3 * P_HID], f16, kind="ExternalInput")
    cw = 84 if gate_only else 168
    wm_d = nc.dram_tensor("wmat", [P_HID, cw], f16, kind="ExternalInput")
    bias_d = nc.dram_tensor("biases", [P_HID, 4], f32, kind="ExternalInput")
    # output in [img, row, c, w]; host transposes back to [img, c, row, w]
    out_d = nc.dram_tensor("out", [IMG_PER_CORE, H, C, W], f32, kind="ExternalOutput")

    ps_bufs = 2 if gate_only else 1

    with tile.TileContext(nc) as tc:
        with (
            tc.tile_pool(name="consts", bufs=1) as cpool,
            tc.tile_pool(name="win", bufs=4) as win_pool,
            tc.tile_pool(name="outp", bufs=4) as out_pool,
            tc.tile_pool(name="hid", bufs=4) as hid_pool,
            tc.tile_pool(name="gp", bufs=4) as g_pool,
            tc.tile_pool(name="ps1", bufs=ps_bufs, space=bass.MemorySpace.PSUM) as ps1_pool,
            tc.tile_pool(name="ps3", bufs=ps_bufs, space=bass.MemorySpace.PSUM) as ps3_pool,
        ):
            btile = cpool.tile([P_WIN, 3 * P_HID], f16)
            nc.scalar.dma_start(out=btile[:, :], in_=bm_d[:, :])
            wmt = cpool.tile([P_HID, cw], f16)
            nc.scalar.dma_start(out=wmt[:, :], in_=wm_d[:, :])
            biast = cpool.tile([P_HID, 4], f32)
            nc.scalar.dma_start(out=biast[:, :], in_=bias_d[:, :])
            bt = btile[0:P_WIN, 0 : 3 * P_HID]
            w3t = wmt[0:P_HID, 0:84]
            b1t = biast[0:P_HID, 0:1]
            b3t = biast[0:P_OUT, 1:2]
            tct = biast[0:P_OUT, 2:3]
            if not gate_only:
                w2t = wmt[0:P_HID, 84:168]
                b2t = biast[0:P_OUT, 3:4]

            for i in range(IMG_PER_CORE):
                sizes = [1, 4, 5, 5, 5, 5] if i == 0 else [5] * 5
                b0 = 0
                for k, sz in enumerate(sizes):
                    r0 = HO * b0  # first output row of chunk
                    is_last = b0 + sz == NBLK
                    win = win_pool.tile([P_WIN, CHUNK * WP], f16)
                    win3 = win.rearrange("p (b w) -> p b w", w=WP)
                    # aligned rows h=0..20 of each block <- padded rows 21b+1+h
                    # dst partitions 0..84 contiguous; src (h c) rows merge
                    nc.sync.dma_start(
                        out=win3[0:P_OUT, 0:sz, :],
                        in_=xp_d[i, r0 + 1 : r0 + 1 + HO * sz, :, :].rearrange(
                            "(b h) c w -> (h c) b w", h=HO
                        ),
                    )
                    # halo row -1 (slot 21, p84..88) <- padded row 21b
                    eng1 = nc.gpsimd if (i == 0 and k == 0) else nc.sync
                    eng2 = nc.scalar if (i == 0 and k == 0) else nc.sync
                    eng1.dma_start(
                        out=win3[P_OUT : P_OUT + 4, 0:sz, :],
                        in_=xp_d[i, r0 : r0 + (sz - 1) * HO + 1 : HO, :, :].rearrange(
                            "b c w -> c b w"
                        ),
                    )
                    # halo row 21 (slot 22, p88..92) <- padded row 21b+22
                    eng2.dma_start(
                        out=win3[P_OUT + 4 : P_OUT + 8, 0:sz, :],
                        in_=xp_d[
                            i, r0 + 22 : r0 + 22 + (sz - 1) * HO + 1 : HO, :, :
                        ].rearrange("b c w -> c b w"),
                    )

                    outt = out_pool.tile([P_OUT, CHUNK * W], f32)

                    for g0, ng in GROUPS_BY_SZ[sz]:
                        nf = ng * W
                        ps1 = ps1_pool.tile([P_HID, nf], f32, tag="ps1")
                        for t in range(3):  # kx taps; dx = t-1
                            for j in range(ng):
                                nc.tensor.matmul(
                                    ps1[:, j * W : (j + 1) * W],
                                    bt[0:P_WIN, t * P_HID : (t + 1) * P_HID],
                                    win[0:P_WIN, (g0 + j) * WP + t :
                                        (g0 + j) * WP + t + W],
                                    start=(t == 0),
                                    stop=(t == 2),
                                )
                        hid = hid_pool.tile([P_HID, nf], f16, tag="hid")
                        nc.scalar.activation(
                            hid[:, :], ps1[:, :], AF.Relu, bias=b1t[:, 0:1]
                        )
                        ps3 = ps3_pool.tile([P_OUT, nf], f32, tag="ps3")
                        for j in range(ng):
                            nc.tensor.matmul(
                                ps3[:, j * W : (j + 1) * W], w3t[:, :],
                                hid[:, j * W : (j + 1) * W],
                                start=True, stop=True,
                            )
                        g = g_pool.tile([P_OUT, nf], f32, tag="g")
                        nc.scalar.activation(
                            g[:, :], ps3[:, :], AF.Sigmoid, bias=b3t[:, 0:1]
                        )
                        # x view over the group's blocks: [84, ng, 512]
                        xa = win.rearrange("p (b w) -> p b w", w=WP)[
                            0:P_OUT, g0 : g0 + ng, 1 : 1 + W
                        ]
                        o3 = outt.rearrange("p (b w) -> p b w", w=W)[
                            0:P_OUT, g0 : g0 + ng, :
                        ]
                        g3 = g.rearrange("p (b w) -> p b w", w=W)
                        if gate_only:
                            # out = (x - tc)*g + tc     (all on VectorE)
                            nc.vector.scalar_tensor_tensor(
                                o3, xa, tct[:, 0:1], g3,
                                op0=ALU.subtract, op1=ALU.mult,
                            )
                            nc.vector.tensor_scalar(
                                o3, o3, tct[:, 0:1], None, ALU.add
                            )
                        else:
                            ps3b = ps3_pool.tile([P_OUT, nf], f32, tag="ps3")
                            for j in range(ng):
                                nc.tensor.matmul(
                                    ps3b[:, j * W : (j + 1) * W], w2t[:, :],
                                    hid[:, j * W : (j + 1) * W],
                                    start=True, stop=True,
                                )
                            yt = g_pool.tile([P_OUT, nf], f32, tag="yt")
                            nc.scalar.activation(
                                yt[:, :], ps3b[:, :], AF.Tanh, bias=b2t[:, 0:1]
                            )
                            y3 = yt.rearrange("p (b w) -> p b w", w=W)
                            d = g_pool.tile([P_OUT, nf], f32, tag="d")
                            d3 = d.rearrange("p (b w) -> p b w", w=W)
                            nc.vector.tensor_sub(d3, xa, y3)
                            nc.vector.tensor_mul(o3, d3, g3)
                            nc.vector.tensor_add(o3, o3, y3)

                    # store chunk
                    outt3 = outt.rearrange("p (b w) -> p b w", w=W)
                    # out-DMAs go on GpSimd (SWDGE): they wait on the blend,
                    # and on the sync HWDGE ring that wait would block the
                    # NEXT chunk's window-DMA issue and stall the PE.
                    nb = sz if not is_last else sz - 1
                    nc.gpsimd.dma_start(
                        out=out_d[i, r0 : r0 + nb * HO, :, :].rearrange(
                            "(b h) c w -> (h c) b w", h=HO
                        ),
                        in_=outt3[:, 0:nb, :],
                    )
                    if is_last:
                        nc.gpsimd.dma_start(
                            out=out_d[i, r0 + nb * HO :, :, :].rearrange(
                                "h c w -> (h c) w"
                            ),
                            in_=outt[0 : LAST_ROWS * C, nb * W : sz * W],
                        )
                    b0 += sz
    nc.compile()
    return nc


def _host_consts(filters, w1_w, w1_b, w2_w, w2_b, w3_w, w3_b, gate_only):
    filters = np.asarray(filters, np.float64)
    w1_w = np.asarray(w1_w, np.float64)
    # fused 4->6ch 3x3 conv kernel
    kf = np.einsum("ocf,fyx->ocyx", w1_w.reshape(O_HID, C, 4), filters)
    bmats = np.zeros((3, P_WIN, P_HID), np.float64)
    for kx in range(3):
        for ho in range(HO):
            for ky in range(3):
                r = ho - 1 + ky
                h_idx = HO if r == -1 else (HO + 1 if r == HO else r)
                for c in range(C):
                    bmats[kx, h_idx * 4 + c, ho * O_HID :
                          ho * O_HID + O_HID] = kf[:, c, ky, kx]
    w3g = np.zeros((P_HID, P_OUT), np.float64)
    w2g = np.zeros((P_HID, P_OUT), np.float64)
    for ho in range(HO):
        for o in range(O_HID):
            for c in range(C):
                w3g[ho * O_HID + o, ho * 4 + c] = np.asarray(w3_w, np.float64)[c, o]
                w2g[ho * O_HID + o, ho * 4 + c] = np.asarray(w2_w, np.float64)[c, o]
    b1v = np.tile(np.asarray(w1_b, np.float64), HO)
    b3v = np.tile(np.asarray(w3_b, np.float64), HO)
    b2v = np.tile(np.asarray(w2_b, np.float64), HO)
    tcv = np.tanh(b2v)
    cw = 84 if gate_only else 168
    packed = np.zeros((P_HID, cw), np.float64)
    packed[:, 0:84] = w3g
    if not gate_only:
        packed[:, 84:168] = w2g
    biases = np.zeros((P_HID, 4), np.float64)
    biases[:, 0] = b1v
    biases[0:P_OUT, 1] = b3v
    biases[0:P_OUT, 2] = tcv
    biases[0:P_OUT, 3] = b2v
    return {
        "bmat": np.ascontiguousarray(
            bmats.transpose(1, 0, 2).reshape(P_WIN, 378), np.float16
        ),
        "wmat": np.ascontiguousarray(packed, np.float16),
        "biases": np.ascontiguousarray(biases, np.float32),
    }


def _run(inputs, trace=False, trace_kwargs=None):
    from concourse.bass_utils import run_bass_kernel_spmd

    x = np.asarray(inputs["x"], np.float32)
    w2_w = np.asarray(inputs["w2_w"], np.float32)
    gate_only = bool(np.all(w2_w == 0.0))
    consts = _host_consts(
        inputs["filters"], inputs["w1_w"], inputs["w1_b"], w2_w,
        inputs["w2_b"], inputs["w3_w"], inputs["w3_b"], gate_only,
    )
    if gate_only not in _PROG_CACHE:
        _PROG_CACHE[gate_only] = _build_program(gate_only)
    nc = _PROG_CACHE[gate_only]

    xpad = np.pad(x, ((0, 0), (0, 0), (1, 14), (1, 1)), mode="wrap")
    xq = np.ascontiguousarray(
        xpad.transpose(0, 2, 1, 3).astype(np.float16)
    )  # [B, HP, C, WP] fp16
    in_maps = []
    for i in range(N_CORES):
        m = dict(consts)
        m["xp"] = np.ascontiguousarray(
            xq[i * IMG_PER_CORE : (i + 1) * IMG_PER_CORE]
        )
        in_maps.append(m)
    kw = {}
    if trace:
        kw = {"trace": True, **(trace_kwargs or {})}
    res = run_bass_kernel_spmd(nc, in_maps, list(range(N_CORES)), **kw)
    oq = np.concatenate([res.results[i]["out"] for i in range(N_CORES)], axis=0)
    out = np.ascontiguousarray(oq.transpose(0, 2, 1, 3))  # -> [B, C, H, W]
    return out.astype(np.float32), res


def kernel(**inputs) -> np.ndarray:
    out, _ = _run(inputs, trace=False)
    return out


# revision 27
# speedup vs baseline: 1.1964x; 1.0742x over previous
"""Trainium2 Bass kernel for nn_BaselineNCA (dense_cnn, memory-bound).

Network (per image):
    y   = perchannel_conv(x, filters)          # 4 fixed 3x3 filters, circular pad
    hid = relu(w1 @ y + b1)                    # 16 -> 6 channels (1x1 conv)
    y_u = tanh(w2 @ hid + b2)                  # 6 -> 4
    g   = sigmoid(w3 @ hid + b3)               # 6 -> 4
    out = x*g + (1-g)*y_u

Strategy: pure data parallel, 2 images per core on 8 cores.  The
per-channel conv + first 1x1 conv fold into one 4->6ch 3x3 conv whose
weights are baked host-side into banded stationary matrices; the H
direction of the conv and the channel contraction both run on the
TensorEngine (float32r, single-pass) as PSUM-accumulated matmuls, one
per kx shift, reading the same SBUF window at +-1 column offsets.  x is
circularly pre-padded on the host so device DMAs are simple strided
reads.  Elementwise work (relu/sigmoid on ScalarE, gated blend on
VectorE) is merged over pairs of blocks to amortize per-op overhead;
GpSimd is never used (its SW-emulated tensor ops are ~9us and hold the
DVE shared SBUF port).

Layouts (per 21-output-row block, W=512 free dim):
    window  [92,514]  p = h*4+c, h=0..20 aligned rows, p84..88 row -1,
                      p88..92 row 21 (halo rows parked after the aligned
                      rows so blend operands share partition base 0)
    psum1   [126,N]   m = ho*6+o   (hidden pre-act)
    psum3   [84,N]    m = ho*4+c   (gate / y_upd pre-act)
    out     [84,N]    m = ho*4+c
"""

import numpy as np

B, C, H, W = 16, 4, 512, 512
N_CORES = 8
IMG_PER_CORE = B // N_CORES
HO = 21                 # output rows per block
NBLK = 25               # 24 full blocks + 1 block of 8 valid rows
LAST_ROWS = H - (NBLK - 1) * HO  # 8
CHUNK = 5               # blocks per DMA chunk
NCHUNK = NBLK // CHUNK  # 5
GROUPS_BY_SZ = {1: [(0, 1)], 4: [(0, 2), (2, 2)],
                5: [(0, 2), (2, 2), (4, 1)]}  # (start, nblocks) merge groups
HP = H + 15             # padded rows: 1 top + 14 bottom wrap
WP = W + 2              # padded cols
O_HID = 6
P_WIN = 92              # 23 window rows * 4 ch
P_HID = HO * O_HID      # 126
P_OUT = HO * C          # 84

_PROG_CACHE: dict = {}


def _patch_ldw_opt():
    """walrus ships with --enable-ldw-opt=false hardcoded; redundant
    LDWEIGHTS (same stationary back-to-back) pace the PE here.  Rewrite
    the flag on the walrus command line.  Correctness is re-verified
    against the reference output shape/values on every run."""
    import concourse.bass_utils as bu

    if getattr(bu, "_ldw_opt_patched", False):
        return
    orig = bu.run_command

    def run_command(argv, **kwargs):
        argv = [
            "--enable-ldw-opt=true" if a == "--enable-ldw-opt=false" else a
            for a in argv
        ]
        return orig(argv, **kwargs)

    bu.run_command = run_command
    bu._ldw_opt_patched = True


def _build_program(gate_only: bool):
    import concourse.bacc as bacc
    import concourse.bass as bass
    import concourse.mybir as mybir
    from concourse import tile

    # note: _patch_ldw_opt() helps fp32/f32r builds but is incompatible
    # with the standalone LDWEIGHTS that bf16 matmuls emit; bf16 weight
    # loads use FWL and are cheap, so leave ldw-opt off here.

    f32 = mybir.dt.float32
    f32r = mybir.dt.float32r
    AF = mybir.ActivationFunctionType
    ALU = mybir.AluOpType

    nc = bacc.Bacc(None, target_bir_lowering=False)

    # x pre-padded AND pre-transposed on host to [img, row, c, w] so the
    # window DMAs are single partition-contiguous transfers
    f16 = mybir.dt.float16
    # whole matmul path in fp16: full PE rate + FWL weight loads like
    # bf16, but 10-bit mantissa (values here are O(10) -- no range risk)
    xp_d = nc.dram_tensor("xp", [IMG_PER_CORE, HP, C, WP], f16, kind="ExternalInput")
    bm_d = nc.dram_tensor("bmat", [P_WIN, 3 * P_HID], f16, kind="ExternalInput")
    cw = 84 if gate_only else 168
    wm_d = nc.dram_tensor("wmat", [P_HID, cw], f16, kind="ExternalInput")
    bias_d = nc.dram_tensor("biases", [P_HID, 4], f32, kind="ExternalInput")
    # output in [img, row, c, w] fp16; host upcasts + transposes back
    out_d = nc.dram_tensor("out", [IMG_PER_CORE, H, C, W], f16, kind="ExternalOutput")

    ps_bufs = 2 if gate_only else 1

    with tile.TileContext(nc) as tc:
        with (
            tc.tile_pool(name="consts", bufs=1) as cpool,
            tc.tile_pool(name="win", bufs=4) as win_pool,
            tc.tile_pool(name="outp", bufs=4) as out_pool,
            tc.tile_pool(name="hid", bufs=4) as hid_pool,
            tc.tile_pool(name="gp", bufs=4) as g_pool,
            tc.tile_pool(name="ps1", bufs=ps_bufs, space=bass.MemorySpace.PSUM) as ps1_pool,
            tc.tile_pool(name="ps3", bufs=ps_bufs, space=bass.MemorySpace.PSUM) as ps3_pool,
        ):
            btile = cpool.tile([P_WIN, 3 * P_HID], f16)
            nc.sync.dma_start(out=btile[:, :], in_=bm_d[:, :])
            wmt = cpool.tile([P_HID, cw], f16)
            biast = cpool.tile([P_HID, 4], f32)
            bt = btile[0:P_WIN, 0 : 3 * P_HID]
            w3t = wmt[0:P_HID, 0:84]
            b1t = biast[0:P_HID, 0:1]
            b3t = biast[0:P_OUT, 1:2]
            tct = biast[0:P_OUT, 2:3]
            if not gate_only:
                w2t = wmt[0:P_HID, 84:168]
                b2t = biast[0:P_OUT, 3:4]

            for i in range(IMG_PER_CORE):
                sizes = [1, 4, 5, 5, 5, 5] if i == 0 else [5] * 5
                b0 = 0
                for k, sz in enumerate(sizes):
                    r0 = HO * b0  # first output row of chunk
                    is_last = b0 + sz == NBLK
                    win = win_pool.tile([P_WIN, CHUNK * WP], f16)
                    win3 = win.rearrange("p (b w) -> p b w", w=WP)
                    # aligned rows h=0..20 of each block <- padded rows 21b+1+h
                    # dst partitions 0..84 contiguous; src (h c) rows merge
                    nc.sync.dma_start(
                        out=win3[0:P_OUT, 0:sz, :],
                        in_=xp_d[i, r0 + 1 : r0 + 1 + HO * sz, :, :].rearrange(
                            "(b h) c w -> (h c) b w", h=HO
                        ),
                    )
                    # halo row -1 (slot 21, p84..88) <- padded row 21b
                    eng1 = nc.gpsimd if (i == 0 and k == 0) else nc.sync
                    eng2 = nc.scalar if (i == 0 and k == 0) else nc.sync
                    eng1.dma_start(
                        out=win3[P_OUT : P_OUT + 4, 0:sz, :],
                        in_=xp_d[i, r0 : r0 + (sz - 1) * HO + 1 : HO, :, :].rearrange(
                            "b c w -> c b w"
                        ),
                    )
                    # halo row 21 (slot 22, p88..92) <- padded row 21b+22
                    eng2.dma_start(
                        out=win3[P_OUT + 4 : P_OUT + 8, 0:sz, :],
                        in_=xp_d[
                            i, r0 + 22 : r0 + 22 + (sz - 1) * HO + 1 : HO, :, :
                        ].rearrange("b c w -> c b w"),
                    )

                    if i == 0 and k == 0:
                        nc.scalar.dma_start(out=wmt[:, :], in_=wm_d[:, :])
                        nc.scalar.dma_start(out=biast[:, :], in_=bias_d[:, :])

                    outt = out_pool.tile([P_OUT, CHUNK * W], f16)

                    for gi, (g0, ng) in enumerate(GROUPS_BY_SZ[sz]):
                        nf = ng * W
                        ps1 = ps1_pool.tile([P_HID, nf], f32, tag="ps1")
                        for t in range(3):  # kx taps; dx = t-1
                            for j in range(ng):
                                nc.tensor.matmul(
                                    ps1[:, j * W : (j + 1) * W],
                                    bt[0:P_WIN, t * P_HID : (t + 1) * P_HID],
                                    win[0:P_WIN, (g0 + j) * WP + t :
                                        (g0 + j) * WP + t + W],
                                    start=(t == 0),
                                    stop=(t == 2),
                                )
                        hid = hid_pool.tile([P_HID, nf], f16, tag="hid")
                        if gi == 1:
                            # balance: route this group's relu to VectorE
                            nc.vector.tensor_scalar(
                                hid[:, :], ps1[:, :], b1t[:, 0:1], 0.0,
                                ALU.add, ALU.max,
                            )
                        else:
                            nc.scalar.activation(
                                hid[:, :], ps1[:, :], AF.Relu, bias=b1t[:, 0:1]
                            )
                        ps3 = ps3_pool.tile([P_OUT, nf], f32, tag="ps3")
                        for j in range(ng):
                            nc.tensor.matmul(
                                ps3[:, j * W : (j + 1) * W], w3t[:, :],
                                hid[:, j * W : (j + 1) * W],
                                start=True, stop=True,
                            )
                        g = g_pool.tile([P_OUT, nf], f16, tag="g")
                        nc.scalar.activation(
                            g[:, :], ps3[:, :], AF.Sigmoid, bias=b3t[:, 0:1]
                        )
                        # x view over the group's blocks: [84, ng, 512]
                        xa = win.rearrange("p (b w) -> p b w", w=WP)[
                            0:P_OUT, g0 : g0 + ng, 1 : 1 + W
                        ]
                        o3 = outt.rearrange("p (b w) -> p b w", w=W)[
                            0:P_OUT, g0 : g0 + ng, :
                        ]
                        g3 = g.rearrange("p (b w) -> p b w", w=W)
                        if gate_only:
                            # out = (x - tc)*g + tc     (all on VectorE)
                            nc.vector.scalar_tensor_tensor(
                                o3, xa, tct[:, 0:1], g3,
                                op0=ALU.subtract, op1=ALU.mult,
                            )
                            nc.vector.tensor_scalar(
                                o3, o3, tct[:, 0:1], None, ALU.add
                            )
                        else:
                            ps3b = ps3_pool.tile([P_OUT, nf], f32, tag="ps3")
                            for j in range(ng):
                                nc.tensor.matmul(
                                    ps3b[:, j * W : (j + 1) * W], w2t[:, :],
                                    hid[:, j * W : (j + 1) * W],
                                    start=True, stop=True,
                                )
                            yt = g_pool.tile([P_OUT, nf], f32, tag="yt")
                            nc.scalar.activation(
                                yt[:, :], ps3b[:, :], AF.Tanh, bias=b2t[:, 0:1]
                            )
                            y3 = yt.rearrange("p (b w) -> p b w", w=W)
                            d = g_pool.tile([P_OUT, nf], f32, tag="d")
                            d3 = d.rearrange("p (b w) -> p b w", w=W)
                            nc.vector.tensor_sub(d3, xa, y3)
                            nc.vector.tensor_mul(o3, d3, g3)
                            nc.vector.tensor_add(o3, o3, y3)

                    # store chunk
                    outt3 = outt.rearrange("p (b w) -> p b w", w=W)
                    # out-DMAs go on GpSimd (SWDGE): they wait on the blend,
                    # and on the sync HWDGE ring that wait would block the
                    # NEXT chunk's window-DMA issue and stall the PE.
                    nb = sz if not is_last else sz - 1
                    out_eng = nc.sync if (i == IMG_PER_CORE - 1 and k >= 3) else nc.gpsimd
                    out_eng.dma_start(
                        out=out_d[i, r0 : r0 + nb * HO, :, :].rearrange(
                            "(b h) c w -> (h c) b w", h=HO
                        ),
                        in_=outt3[:, 0:nb, :],
                    )
                    if is_last:
                        out_eng.dma_start(
                            out=out_d[i, r0 + nb * HO :, :, :].rearrange(
                                "h c w -> (h c) w"
                            ),
                            in_=outt[0 : LAST_ROWS * C, nb * W : sz * W],
                        )
                    b0 += sz
    nc.compile()
    return nc


def _host_consts(filters, w1_w, w1_b, w2_w, w2_b, w3_w, w3_b, gate_only):
    filters = np.asarray(filters, np.float64)
    w1_w = np.asarray(w1_w, np.float64)
    # fused 4->6ch 3x3 conv kernel
    kf = np.einsum("ocf,fyx->ocyx", w1_w.reshape(O_HID, C, 4), filters)
    bmats = np.zeros((3, P_WIN, P_HID), np.float64)
    for kx in range(3):
        for ho in range(HO):
            for ky in range(3):
                r = ho - 1 + ky
                h_idx = HO if r == -1 else (HO + 1 if r == HO else r)
                for c in range(C):
                    bmats[kx, h_idx * 4 + c, ho * O_HID :
                          ho * O_HID + O_HID] = kf[:, c, ky, kx]
    w3g = np.zeros((P_HID, P_OUT), np.float64)
    w2g = np.zeros((P_HID, P_OUT), np.float64)
    for ho in range(HO):
        for o in range(O_HID):
            for c in range(C):
                w3g[ho * O_HID + o, ho * 4 + c] = np.asarray(w3_w, np.float64)[c, o]
                w2g[ho * O_HID + o, ho * 4 + c] = np.asarray(w2_w, np.float64)[c, o]
    b1v = np.tile(np.asarray(w1_b, np.float64), HO)
    b3v = np.tile(np.asarray(w3_b, np.float64), HO)
    b2v = np.tile(np.asarray(w2_b, np.float64), HO)
    tcv = np.tanh(b2v)
    cw = 84 if gate_only else 168
    packed = np.zeros((P_HID, cw), np.float64)
    packed[:, 0:84] = w3g
    if not gate_only:
        packed[:, 84:168] = w2g
    biases = np.zeros((P_HID, 4), np.float64)
    biases[:, 0] = b1v
    biases[0:P_OUT, 1] = b3v
    biases[0:P_OUT, 2] = tcv
    biases[0:P_OUT, 3] = b2v
    return {
        "bmat": np.ascontiguousarray(
            bmats.transpose(1, 0, 2).reshape(P_WIN, 378), np.float16
        ),
        "wmat": np.ascontiguousarray(packed, np.float16),
        "biases": np.ascontiguousarray(biases, np.float32),
    }


def _run(inputs, trace=False, trace_kwargs=None):
    from concourse.bass_utils import run_bass_kernel_spmd

    x = np.asarray(inputs["x"], np.float32)
    w2_w = np.asarray(inputs["w2_w"], np.float32)
    gate_only = bool(np.all(w2_w == 0.0))
    consts = _host_consts(
        inputs["filters"], inputs["w1_w"], inputs["w1_b"], w2_w,
        inputs["w2_b"], inputs["w3_w"], inputs["w3_b"], gate_only,
    )
    if gate_only not in _PROG_CACHE:
        _PROG_CACHE[gate_only] = _build_program(gate_only)
    nc = _PROG_CACHE[gate_only]

    xpad = np.pad(x, ((0, 0), (0, 0), (1, 14), (1, 1)), mode="wrap")
    xq = np.ascontiguousarray(
        xpad.transpose(0, 2, 1, 3).astype(np.float16)
    )  # [B, HP, C, WP] fp16
    in_maps = []
    for i in range(N_CORES):
        m = dict(consts)
        m["xp"] = np.ascontiguousarray(
            xq[i * IMG_PER_CORE : (i + 1) * IMG_PER_CORE]
        )
        in_maps.append(m)
    kw = {}
    if trace:
        kw = {"trace": True, **(trace_kwargs or {})}
    res = run_bass_kernel_spmd(nc, in_maps, list(range(N_CORES)), **kw)
    oq = np.concatenate([res.results[i]["out"] for i in range(N_CORES)], axis=0)
    out = np.ascontiguousarray(
        oq.astype(np.float32).transpose(0, 2, 1, 3)
    )  # -> [B, C, H, W] fp32
    return out, res


def kernel(**inputs) -> np.ndarray:
    out, _ = _run(inputs, trace=False)
    return out


# revision 29
# speedup vs baseline: 1.2448x; 1.0405x over previous
"""Trainium2 Bass kernel for nn_BaselineNCA (dense_cnn, memory-bound).

Network (per image):
    y   = perchannel_conv(x, filters)          # 4 fixed 3x3 filters, circular pad
    hid = relu(w1 @ y + b1)                    # 16 -> 6 channels (1x1 conv)
    y_u = tanh(w2 @ hid + b2)                  # 6 -> 4
    g   = sigmoid(w3 @ hid + b3)               # 6 -> 4
    out = x*g + (1-g)*y_u

Strategy: pure data parallel, 2 images per core on 8 cores.  The
per-channel conv + first 1x1 conv fold into one 4->6ch 3x3 conv whose
weights are baked host-side into banded stationary matrices; the H
direction of the conv and the channel contraction both run on the
TensorEngine (float32r, single-pass) as PSUM-accumulated matmuls, one
per kx shift, reading the same SBUF window at +-1 column offsets.  x is
circularly pre-padded on the host so device DMAs are simple strided
reads.  Elementwise work (relu/sigmoid on ScalarE, gated blend on
VectorE) is merged over pairs of blocks to amortize per-op overhead;
GpSimd is never used (its SW-emulated tensor ops are ~9us and hold the
DVE shared SBUF port).

Layouts (per 21-output-row block, W=512 free dim):
    window  [92,514]  p = h*4+c, h=0..20 aligned rows, p84..88 row -1,
                      p88..92 row 21 (halo rows parked after the aligned
                      rows so blend operands share partition base 0)
    psum1   [126,N]   m = ho*6+o   (hidden pre-act)
    psum3   [84,N]    m = ho*4+c   (gate / y_upd pre-act)
    out     [84,N]    m = ho*4+c
"""

import numpy as np

B, C, H, W = 16, 4, 512, 512
N_CORES = 8
IMG_PER_CORE = B // N_CORES
HO = 21                 # output rows per block
NBLK = 25               # 24 full blocks + 1 block of 8 valid rows
LAST_ROWS = H - (NBLK - 1) * HO  # 8
CHUNK = 5               # blocks per DMA chunk
NCHUNK = NBLK // CHUNK  # 5
GROUPS_BY_SZ = {1: [(0, 1)], 4: [(0, 2), (2, 2)],
                5: [(0, 2), (2, 2), (4, 1)]}  # (start, nblocks) merge groups
HP = H + 15             # padded rows: 1 top + 14 bottom wrap
WP = W + 2              # padded cols
O_HID = 6
P_WIN = 92              # 23 window rows * 4 ch
P_HID = HO * O_HID      # 126
P_OUT = HO * C          # 84

_PROG_CACHE: dict = {}


def _patch_ldw_opt():
    """walrus ships with --enable-ldw-opt=false hardcoded; redundant
    LDWEIGHTS (same stationary back-to-back) pace the PE here.  Rewrite
    the flag on the walrus command line.  Correctness is re-verified
    against the reference output shape/values on every run."""
    import concourse.bass_utils as bu

    if getattr(bu, "_ldw_opt_patched", False):
        return
    orig = bu.run_command

    def run_command(argv, **kwargs):
        argv = [
            "--enable-ldw-opt=true" if a == "--enable-ldw-opt=false" else a
            for a in argv
        ]
        return orig(argv, **kwargs)

    bu.run_command = run_command
    bu._ldw_opt_patched = True


def _build_program(gate_only: bool):
    import concourse.bacc as bacc
    import concourse.bass as bass
    import concourse.mybir as mybir
    from concourse import tile

    # note: _patch_ldw_opt() helps fp32/f32r builds but is incompatible
    # with the standalone LDWEIGHTS that bf16 matmuls emit; bf16 weight
    # loads use FWL and are cheap, so leave ldw-opt off here.

    f32 = mybir.dt.float32
    f32r = mybir.dt.float32r
    AF = mybir.ActivationFunctionType
    ALU = mybir.AluOpType

    nc = bacc.Bacc(None, target_bir_lowering=False)

    # x pre-padded AND pre-transposed on host to [img, row, c, w] so the
    # window DMAs are single partition-contiguous transfers
    f16 = mybir.dt.float16
    # whole matmul path in fp16: full PE rate + FWL weight loads like
    # bf16, but 10-bit mantissa (values here are O(10) -- no range risk)
    xp_d = nc.dram_tensor("xp", [IMG_PER_CORE, HP, C, WP], f16, kind="ExternalInput")
    bm_d = nc.dram_tensor("bmat", [P_WIN, 3 * P_HID], f16, kind="ExternalInput")
    cw = 84 if gate_only else 168
    wm_d = nc.dram_tensor("wmat", [P_HID, cw], f16, kind="ExternalInput")
    bias_d = nc.dram_tensor("biases", [P_HID, 4], f32, kind="ExternalInput")
    # output in [img, row, c, w] fp16; host upcasts + transposes back
    out_d = nc.dram_tensor("out", [IMG_PER_CORE, H, C, W], f16, kind="ExternalOutput")

    ps_bufs = 2 if gate_only else 1

    with tile.TileContext(nc) as tc:
        with (
            tc.tile_pool(name="consts", bufs=1) as cpool,
            tc.tile_pool(name="win", bufs=4) as win_pool,
            tc.tile_pool(name="outp", bufs=4) as out_pool,
            tc.tile_pool(name="hid", bufs=4) as hid_pool,
            tc.tile_pool(name="gp", bufs=4) as g_pool,
            tc.tile_pool(name="ps1", bufs=ps_bufs, space=bass.MemorySpace.PSUM) as ps1_pool,
            tc.tile_pool(name="ps3", bufs=ps_bufs, space=bass.MemorySpace.PSUM) as ps3_pool,
        ):
            btile = cpool.tile([P_WIN, 3 * P_HID], f16)
            nc.sync.dma_start(out=btile[:, :], in_=bm_d[:, :])
            wmt = cpool.tile([P_HID, cw], f16)
            biast = cpool.tile([P_HID, 4], f32)
            bt = btile[0:P_WIN, 0 : 3 * P_HID]
            w3t = wmt[0:P_HID, 0:84]
            b1t = biast[0:P_HID, 0:1]
            b3t = biast[0:P_OUT, 1:2]
            tct = biast[0:P_OUT, 2:3]
            if not gate_only:
                w2t = wmt[0:P_HID, 84:168]
                b2t = biast[0:P_OUT, 3:4]

            for i in range(IMG_PER_CORE):
                sizes = [1, 4, 5, 5, 5, 5] if i == 0 else [5, 5, 5, 5, 4, 1]
                b0 = 0
                for k, sz in enumerate(sizes):
                    r0 = HO * b0  # first output row of chunk
                    is_last = b0 + sz == NBLK
                    win = win_pool.tile([P_WIN, CHUNK * WP], f16)
                    win3 = win.rearrange("p (b w) -> p b w", w=WP)
                    # aligned rows h=0..20 of each block <- padded rows 21b+1+h
                    # dst partitions 0..84 contiguous; src (h c) rows merge
                    nc.sync.dma_start(
                        out=win3[0:P_OUT, 0:sz, :],
                        in_=xp_d[i, r0 + 1 : r0 + 1 + HO * sz, :, :].rearrange(
                            "(b h) c w -> (h c) b w", h=HO
                        ),
                    )
                    # halo row -1 (slot 21, p84..88) <- padded row 21b
                    eng1 = nc.gpsimd if (i == 0 and k == 0) else nc.sync
                    eng2 = nc.scalar if (i == 0 and k == 0) else nc.sync
                    eng1.dma_start(
                        out=win3[P_OUT : P_OUT + 4, 0:sz, :],
                        in_=xp_d[i, r0 : r0 + (sz - 1) * HO + 1 : HO, :, :].rearrange(
                            "b c w -> c b w"
                        ),
                    )
                    # halo row 21 (slot 22, p88..92) <- padded row 21b+22
                    eng2.dma_start(
                        out=win3[P_OUT + 4 : P_OUT + 8, 0:sz, :],
                        in_=xp_d[
                            i, r0 + 22 : r0 + 22 + (sz - 1) * HO + 1 : HO, :, :
                        ].rearrange("b c w -> c b w"),
                    )

                    if i == 0 and k == 0:
                        nc.scalar.dma_start(out=wmt[:, :], in_=wm_d[:, :])
                        nc.scalar.dma_start(out=biast[:, :], in_=bias_d[:, :])

                    outt = out_pool.tile([P_OUT, CHUNK * W], f16)

                    for gi, (g0, ng) in enumerate(GROUPS_BY_SZ[sz]):
                        nf = ng * W
                        ps1 = ps1_pool.tile([P_HID, nf], f32, tag="ps1")
                        for t in range(3):  # kx taps; dx = t-1
                            for j in range(ng):
                                nc.tensor.matmul(
                                    ps1[:, j * W : (j + 1) * W],
                                    bt[0:P_WIN, t * P_HID : (t + 1) * P_HID],
                                    win[0:P_WIN, (g0 + j) * WP + t :
                                        (g0 + j) * WP + t + W],
                                    start=(t == 0),
                                    stop=(t == 2),
                                )
                        hid = hid_pool.tile([P_HID, nf], f16, tag="hid")
                        if gi == 1:
                            # balance: route this group's relu to VectorE
                            nc.vector.tensor_scalar(
                                hid[:, :], ps1[:, :], b1t[:, 0:1], 0.0,
                                ALU.add, ALU.max,
                            )
                        else:
                            nc.scalar.activation(
                                hid[:, :], ps1[:, :], AF.Relu, bias=b1t[:, 0:1]
                            )
                        ps3 = ps3_pool.tile([P_OUT, nf], f32, tag="ps3")
                        for j in range(ng):
                            nc.tensor.matmul(
                                ps3[:, j * W : (j + 1) * W], w3t[:, :],
                                hid[:, j * W : (j + 1) * W],
                                start=True, stop=True,
                            )
                        g = g_pool.tile([P_OUT, nf], f16, tag="g")
                        nc.scalar.activation(
                            g[:, :], ps3[:, :], AF.Sigmoid, bias=b3t[:, 0:1]
                        )
                        # x view over the group's blocks: [84, ng, 512]
                        xa = win.rearrange("p (b w) -> p b w", w=WP)[
                            0:P_OUT, g0 : g0 + ng, 1 : 1 + W
                        ]
                        o3 = outt.rearrange("p (b w) -> p b w", w=W)[
                            0:P_OUT, g0 : g0 + ng, :
                        ]
                        g3 = g.rearrange("p (b w) -> p b w", w=W)
                        if gate_only:
                            # out = (x - tc)*g + tc  -- three DVE ops beat
                            # scalar_tensor_tensor (stt has only a 1x uop;
                            # ts hits 4x and tt 2x in all-fp16)
                            dtl = g_pool.tile([P_OUT, nf], f16, tag="dt")
                            d3 = dtl.rearrange("p (b w) -> p b w", w=W)
                            mtl = g_pool.tile([P_OUT, nf], f16, tag="mt")
                            m3 = mtl.rearrange("p (b w) -> p b w", w=W)
                            nc.vector.tensor_scalar(
                                d3, xa, tct[:, 0:1], None, ALU.subtract
                            )
                            nc.vector.tensor_mul(m3, d3, g3)
                            nc.vector.tensor_scalar(
                                o3, m3, tct[:, 0:1], None, ALU.add
                            )
                        else:
                            ps3b = ps3_pool.tile([P_OUT, nf], f32, tag="ps3")
                            for j in range(ng):
                                nc.tensor.matmul(
                                    ps3b[:, j * W : (j + 1) * W], w2t[:, :],
                                    hid[:, j * W : (j + 1) * W],
                                    start=True, stop=True,
                                )
                            yt = g_pool.tile([P_OUT, nf], f32, tag="yt")
                            nc.scalar.activation(
                                yt[:, :], ps3b[:, :], AF.Tanh, bias=b2t[:, 0:1]
                            )
                            y3 = yt.rearrange("p (b w) -> p b w", w=W)
                            d = g_pool.tile([P_OUT, nf], f32, tag="d")
                            d3 = d.rearrange("p (b w) -> p b w", w=W)
                            nc.vector.tensor_sub(d3, xa, y3)
                            nc.vector.tensor_mul(o3, d3, g3)
                            nc.vector.tensor_add(o3, o3, y3)

                    # store chunk
                    outt3 = outt.rearrange("p (b w) -> p b w", w=W)
                    # out-DMAs go on GpSimd (SWDGE): they wait on the blend,
                    # and on the sync HWDGE ring that wait would block the
                    # NEXT chunk's window-DMA issue and stall the PE.
                    nb = sz if not is_last else sz - 1
                    out_eng = nc.sync if (i == IMG_PER_CORE - 1 and k >= 3) else nc.gpsimd
                    if nb > 0:
                        out_eng.dma_start(
                            out=out_d[i, r0 : r0 + nb * HO, :, :].rearrange(
                                "(b h) c w -> (h c) b w", h=HO
                            ),
                            in_=outt3[:, 0:nb, :],
                        )
                    if is_last:
                        out_eng.dma_start(
                            out=out_d[i, r0 + nb * HO :, :, :].rearrange(
                                "h c w -> (h c) w"
                            ),
                            in_=outt[0 : LAST_ROWS * C, nb * W : sz * W],
                        )
                    b0 += sz
    nc.compile()
    return nc


def _host_consts(filters, w1_w, w1_b, w2_w, w2_b, w3_w, w3_b, gate_only):
    filters = np.asarray(filters, np.float64)
    w1_w = np.asarray(w1_w, np.float64)
    # fused 4->6ch 3x3 conv kernel
    kf = np.einsum("ocf,fyx->ocyx", w1_w.reshape(O_HID, C, 4), filters)
    bmats = np.zeros((3, P_WIN, P_HID), np.float64)
    for kx in range(3):
        for ho in range(HO):
            for ky in range(3):
                r = ho - 1 + ky
                h_idx = HO if r == -1 else (HO + 1 if r == HO else r)
                for c in range(C):
                    bmats[kx, h_idx * 4 + c, ho * O_HID :
                          ho * O_HID + O_HID] = kf[:, c, ky, kx]
    w3g = np.zeros((P_HID, P_OUT), np.float64)
    w2g = np.zeros((P_HID, P_OUT), np.float64)
    for ho in range(HO):
        for o in range(O_HID):
            for c in range(C):
                w3g[ho * O_HID + o, ho * 4 + c] = np.asarray(w3_w, np.float64)[c, o]
                w2g[ho * O_HID + o, ho * 4 + c] = np.asarray(w2_w, np.float64)[c, o]
    b1v = np.tile(np.asarray(w1_b, np.float64), HO)
    b3v = np.tile(np.asarray(w3_b, np.float64), HO)
    b2v = np.tile(np.asarray(w2_b, np.float64), HO)
    tcv = np.tanh(b2v)
    cw = 84 if gate_only else 168
    packed = np.zeros((P_HID, cw), np.float64)
    packed[:, 0:84] = w3g
    if not gate_only:
        packed[:, 84:168] = w2g
    biases = np.zeros((P_HID, 4), np.float64)
    biases[:, 0] = b1v
    biases[0:P_OUT, 1] = b3v
    biases[0:P_OUT, 2] = tcv
    biases[0:P_OUT, 3] = b2v
    return {
        "bmat": np.ascontiguousarray(
            bmats.transpose(1, 0, 2).reshape(P_WIN, 378), np.float16
        ),
        "wmat": np.ascontiguousarray(packed, np.float16),
        "biases": np.ascontiguousarray(biases, np.float32),
    }


def _run(inputs, trace=False, trace_kwargs=None):
    from concourse.bass_utils import run_bass_kernel_spmd

    x = np.asarray(inputs["x"], np.float32)
    w2_w = np.asarray(inputs["w2_w"], np.float32)
    gate_only = bool(np.all(w2_w == 0.0))
    consts = _host_consts(
        inputs["filters"], inputs["w1_w"], inputs["w1_b"], w2_w,
        inputs["w2_b"], inputs["w3_w"], inputs["w3_b"], gate_only,
    )
    if gate_only not in _PROG_CACHE:
        _PROG_CACHE[gate_only] = _build_program(gate_only)
    nc = _PROG_CACHE[gate_only]

    xpad = np.pad(x, ((0, 0), (0, 0), (1, 14), (1, 1)), mode="wrap")
    xq = np.ascontiguousarray(
        xpad.transpose(0, 2, 1, 3).astype(np.float16)
    )  # [B, HP, C, WP] fp16
    in_maps = []
    for i in range(N_CORES):
        m = dict(consts)
        m["xp"] = np.ascontiguousarray(
            xq[i * IMG_PER_CORE : (i + 1) * IMG_PER_CORE]
        )
        in_maps.append(m)
    kw = {}
    if trace:
        kw = {"trace": True, **(trace_kwargs or {})}
    res = run_bass_kernel_spmd(nc, in_maps, list(range(N_CORES)), **kw)
    oq = np.concatenate([res.results[i]["out"] for i in range(N_CORES)], axis=0)
    out = np.ascontiguousarray(
        oq.astype(np.float32).transpose(0, 2, 1, 3)
    )  # -> [B, C, H, W] fp32
    return out, res


def kernel(**inputs) -> np.ndarray:
    out, _ = _run(inputs, trace=False)
    return out
